# revision 9
# baseline (speedup 1.0000x reference)
"""Trainium2 Bass kernel for nn_Concat_26147760898611.

Mean-pool over the word dim of article_concat [256, 2048, 300] and
options_concat [256, 64, 300], concat features -> [256, 600].

Sharding: pure data parallel over batch across 8 NeuronCores
(32 batches per core).  The kernel is HBM-read-bound: 81.1 MB per core
streams through all 16 SDMA engines at their ~27 GB/s per-engine limit
(~421 GB/s aggregate), so the only recoverable time is at the edges.

Per core:
  - each article batch [2048, 300] is DMA'd as one 2.46 MB transfer into
    an SBUF tile [128 partitions, 16 words, 300 feat]; partition p holds
    16 consecutive words (fully contiguous 19.2 KB per partition).
  - the word axis is folded on the VectorEngine; the surviving chunks
    are reduced across the partition dim on the TensorEngine with a
    ones-column selector that routes each batch's sum into its PSUM row
    via a sliding window.  The bulk (batches 0..27 + options) folds
    twice in exact fp32 and uses fp32 matmuls: the near-saturated PE
    keeps the power governor from downclocking, which measurably keeps
    the DMA stream at line rate.
  - the last 4 batches and the final-batch tail chunks use a SINGLE
    fold whose output rounds to bf16 (error ~1e-4 relative on the final
    mean) feeding single-pass bf16 matmuls: half the DVE work and a
    1-fold serial chain exactly where drain latency matters.
  - the article selectors are built on-chip with GpSimd memsets, so no
    DMA precedes the first data DMAs on the Sync queue.
  - batches are split 28/4 across two PSUM banks: bank A (rows 0..27)
    is scaled into SBUF while batches 28..31 still stream; the tail
    bank's 4 rows live at partition 0 of a dedicated [4, 600] tile
    (compute engines need partition-aligned access).  Both output
    stores are issued on Sync AFTER the last data DMA, so the Sync
    sequencer never stalls descriptor generation mid-stream.
  - the last batch is split into geometrically shrinking chunks
    [8,4,2,1,1]; the two single-word chunks are summed by one DVE add
    (fp32+fp32 -> bf16) feeding the final matmul.

Self-contained: hardcodes all shapes; no file reads.
"""

import numpy as np

N_CORES = 8
B = 256  # full batch
BC = B // N_CORES  # 32 batches per core
DIM = 300
AW = 2048  # article words per batch
OW = 64  # options words per batch
P = 128  # SBUF partitions
AWP = AW // P  # 16 article words per partition
TAIL_CHUNKS = [8, 4, 2]  # geometric split of the final batch head
BANK_A = 28  # batches 0..27 -> early-drained PSUM bank; 28..31 -> tail bank
DATA_BUFS = 6
FOLD_BUFS = 3

_CACHE = {}


def _build_nc():
    import concourse.bacc as bacc
    import concourse.mybir as mybir
    import concourse.tile as tile

    f32 = mybir.dt.float32
    bf16 = mybir.dt.bfloat16
    nc = bacc.Bacc("TRN2", target_bir_lowering=False, debug=False)

    art = nc.dram_tensor("article", [BC, AW, DIM], f32, kind="ExternalInput")
    opt = nc.dram_tensor("options", [BC, OW, DIM], f32, kind="ExternalInput")
    sel_o = nc.dram_tensor("sel_o", [P, BC], f32, kind="ExternalInput")
    out = nc.dram_tensor("out", [BC, 2 * DIM], f32, kind="ExternalOutput")

    # [BC, 128, 16, 300]: partition p <- words p*16 .. p*16+15 (contiguous)
    art_r = art.ap().rearrange("b (p w) f -> b p w f", p=P)
    # per-partition word view of the last batch: [128, 16, 300]
    art_last = art.ap()[BC - 1].rearrange("(p w) f -> p w f", p=P)
    # [128, 16, 300]: partition p <- 16 consecutive words of batch p//4
    opt_r = opt.ap().rearrange("b (s q) f -> (b s) q f", s=P // BC)

    with tile.TileContext(nc) as tc:
        with (
            tc.tile_pool(name="const", bufs=1) as cpool,
            tc.tile_pool(name="data", bufs=DATA_BUFS) as dpool,
            tc.tile_pool(name="fold", bufs=FOLD_BUFS) as fpool,
            tc.tile_pool(name="bfold", bufs=FOLD_BUFS) as bpool,
            tc.tile_pool(name="tailf", bufs=1) as tpool,
            tc.tile_pool(name="outp", bufs=1) as opool,
            tc.tile_pool(name="psum", bufs=1, space="PSUM") as ppool,
        ):
            # first Sync-engine instructions: the big data DMAs
            opt_t = dpool.tile([P, AWP, DIM], f32, tag="data")
            nc.sync.dma_start(opt_t[:], opt_r)
            art_t0 = dpool.tile([P, AWP, DIM], f32, tag="data")
            nc.sync.dma_start(art_t0[:], art_r[0])
            art_t1 = dpool.tile([P, AWP, DIM], f32, tag="data")
            nc.sync.dma_start(art_t1[:], art_r[1])

            # options selector (block pattern): tiny DMA, fp32
            sel_of = cpool.tile([P, BC], f32, tag="sel_of")
            nc.sync.dma_start(sel_of[:], sel_o.ap()[:])

            # article selectors built on-chip (GpSimd): ones column at 31
            sel_af = cpool.tile([P, 2 * BC - 1], f32, tag="sel_af")
            nc.gpsimd.memset(sel_af[:], 0.0)
            nc.gpsimd.memset(sel_af[:, BC - 1 : BC], 1.0)
            sel_ab = cpool.tile([P, 2 * BC - 1], bf16, tag="sel_ab")
            nc.gpsimd.memset(sel_ab[:], 0.0)
            nc.gpsimd.memset(sel_ab[:, BC - 1 : BC], 1.0)

            psum_b28 = ppool.tile([BANK_A, DIM], f32, tag="psum_b28")
            psum_b4 = ppool.tile([BC - BANK_A, DIM], f32, tag="psum_b4")
            psum_a = ppool.tile([BANK_A, DIM], f32, tag="psum_a")
            psum_t = ppool.tile([BC - BANK_A, DIM], f32, tag="psum_t")

            out_t = opool.tile([BANK_A, 2 * DIM], f32, tag="out")
            out_tail = opool.tile([BC - BANK_A, 2 * DIM], f32, tag="out_tail")

            def fold2(t, nch):
                """Two folds, both exact fp32: [P,nch,DIM] -> nch//4 chunks."""
                n = nch // 2
                a = fpool.tile([P, n, DIM], f32, tag=f"fold_{nch}")
                nc.vector.tensor_add(a[:], t[:, 0:n, :], t[:, n : 2 * n, :])
                m = n // 2
                bt = bpool.tile([P, m, DIM], f32, tag=f"ffold_{nch}")
                nc.vector.tensor_add(bt[:], a[:, 0:m, :], a[:, m : 2 * m, :])
                return bt, m

            def fold1b(t, nch, pool):
                """One fold, fp32+fp32 -> bf16: [P,nch,DIM] -> nch//2 chunks."""
                m = nch // 2
                bt = pool.tile([P, m, DIM], bf16, tag=f"bfold_{nch}")
                nc.vector.tensor_add(bt[:], t[:, 0:m, :], t[:, m : 2 * m, :])
                return bt, m

            def matmuls(bt, m, sel, psum, first, last):
                for j in range(m):
                    nc.tensor.matmul(
                        psum[:],
                        sel,
                        bt[:, j, :],
                        start=(first and j == 0),
                        stop=(last and j == m - 1),
                    )

            # options (fp32): each folded chunk reduces rows 0..27 and
            # 28..31 via column slices of the block selector
            obt, om = fold2(opt_t, AWP)
            matmuls(obt, om, sel_of[:, 0:BANK_A], psum_b28, True, True)
            matmuls(obt, om, sel_of[:, BANK_A:BC], psum_b4, True, True)
            nc.scalar.mul(out_t[:, DIM : 2 * DIM], psum_b28[:], 1.0 / OW)
            nc.scalar.mul(out_tail[:, DIM : 2 * DIM], psum_b4[:], 1.0 / OW)

            # articles 0..27 (fp32 matmuls) -> bank A (28-wide window)
            for b in range(BANK_A):
                if b == 0:
                    t = art_t0
                elif b == 1:
                    t = art_t1
                else:
                    t = dpool.tile([P, AWP, DIM], f32, tag="data")
                    nc.sync.dma_start(t[:], art_r[b])
                bt, m = fold2(t, AWP)
                matmuls(
                    bt, m, sel_af[:, BC - 1 - b : BC - 1 - b + BANK_A],
                    psum_a, b == 0, b == BANK_A - 1,
                )
            # bank A drain on Scalar while 28..31 stream
            nc.scalar.mul(out_t[:, 0:DIM], psum_a[:], 1.0 / AW)

            # articles 28..30: single fold -> bf16, 8 bf16 matmuls each
            for b in range(BANK_A, BC - 1):
                t = dpool.tile([P, AWP, DIM], f32, tag="data")
                nc.sync.dma_start(t[:], art_r[b])
                bt, m = fold1b(t, AWP, bpool)
                matmuls(
                    bt, m, sel_ab[:, BC - 1 - b + BANK_A : BC - 1 - b + BC],
                    psum_t, b == BANK_A, False,
                )
            # final batch: shrinking chunks, one fold + bf16 matmuls each
            sel_tb = sel_ab[:, BANK_A : BC]
            w0 = 0
            for nch in TAIL_CHUNKS:
                t = dpool.tile([P, nch, DIM], f32, tag="data")
                nc.sync.dma_start(t[:], art_last[:, w0 : w0 + nch, :])
                bt, m = fold1b(t, nch, tpool)
                matmuls(bt, m, sel_tb, psum_t, False, False)
                w0 += nch
            # last two words arrive as two tiny DMAs; one DVE add joins
            # them into a single bf16 chunk for the final matmul
            t_w0 = dpool.tile([P, 1, DIM], f32, tag="data")
            nc.sync.dma_start(t_w0[:], art_last[:, w0 : w0 + 1, :])
            t_w1 = dpool.tile([P, 1, DIM], f32, tag="data")
            nc.sync.dma_start(t_w1[:], art_last[:, w0 + 1 : w0 + 2, :])
            bt_l = tpool.tile([P, 1, DIM], bf16, tag="bfold_last")
            nc.vector.tensor_add(bt_l[:], t_w0[:, 0, :], t_w1[:, 0, :])
            nc.tensor.matmul(psum_t[:], sel_tb, bt_l[:, 0, :], start=False, stop=True)

            # stores: issued on Sync after every data DMA, so descriptor
            # generation for the stream is never blocked by a sem wait
            nc.sync.dma_start(out.ap()[0:BANK_A, :], out_t[:])
            nc.scalar.mul(out_tail[:, 0:DIM], psum_t[:], 1.0 / AW)
            nc.sync.dma_start(out.ap()[BANK_A:BC, :], out_tail[:])

    nc.compile()
    return nc


def get_nc():
    if "nc" not in _CACHE:
        _CACHE["nc"] = _build_nc()
    return _CACHE["nc"]


def _sel_arrays():
    sel_o = np.zeros((P, BC), np.float32)
    sel_o[np.arange(P), np.arange(P) // (P // BC)] = 1.0
    return sel_o


def make_in_maps(article, options):
    article = np.ascontiguousarray(np.asarray(article, dtype=np.float32))
    options = np.ascontiguousarray(np.asarray(options, dtype=np.float32))
    assert article.shape == (B, AW, DIM), article.shape
    assert options.shape == (B, OW, DIM), options.shape
    sel_o = _sel_arrays()
    return [
        {
            "article": article[i * BC : (i + 1) * BC],
            "options": options[i * BC : (i + 1) * BC],
            "sel_o": sel_o,
        }
        for i in range(N_CORES)
    ]


def run_sharded(article, options, **spmd_kwargs):
    from concourse.bass_utils import run_bass_kernel_spmd

    nc = get_nc()
    in_maps = make_in_maps(article, options)
    res = run_bass_kernel_spmd(nc, in_maps, list(range(N_CORES)), **spmd_kwargs)
    full = np.concatenate(
        [res.results[i]["out"] for i in range(N_CORES)], axis=0
    ).astype(np.float32)
    return full, res


def kernel(article_concat, options_concat):
    full, _ = run_sharded(article_concat, options_concat)
    return full
